# revision 7
# baseline (speedup 1.0000x reference)
"""Block-sparse causal attention kernel for Trainium2 (8 NeuronCores).

Problem: B=2, T=2048, H=16, Dqk=Dv=128, fp16, BLOCK 64x64 block mask +
causal, softmax over keys.

Sharding: the 32 (b, h) pairs are split 4-per-core across 8 cores (data +
head parallel); no cross-core communication.

Per-core device algorithm (per (b,h) pair):
  - One contiguous input DMA per pair carrying [Q^T | K^T | V | mask-table]
    as a [128, 6784] fp16 tile (Q^T/K^T pre-transposed on the host; V laid
    out [t mod 128 -> partition, 16 k-tiles, d]).
  - Loop q-groups g (512 queries each), inner k-tile pairs (kt0, kt0+1):
      S^T[n,m] = K_kt @ Q_g^T  on PE (out [128 keys, 512 queries] psum)
      P^T = exp(S^T * 1/sqrt(d))  on ACT (fp16 to SBUF), no max-subtraction
      P^T *= blockmask (broadcast 0/1, DVE); diagonal chunk *= causal 0/1
      O^T[d,m] += V_kt^T @ P^T  on PE (accumulate over kt in psum)
      l[1,m]  += ones^T @ P^T   on PE (softmax denominator)
  - Readout: evacuate O^T (unnormalized, fp32) and l to DRAM; the host does
    the final [d, t] -> [t, d] transpose fused with the 1/l normalization.

The emission is software-pipelined with a lookahead of LOOKAHEAD steps: the
S^T matmuls (and exp/mask) of steps s+1..s+LA are issued before the O/l
matmuls of step s, so the PE streams through the exp/mask latency of the
ACT/DVE chain instead of stalling on it, including across group boundaries.
PSUM: 3 double-bank S tiles + one combined [O | l] 2-bank accumulator per
group, evacuated in a single fp32 DVE copy.

The block mask is applied multiplicatively after exp (scores are O(5), so
exp never overflows), which keeps the program identical across all cores:
the mask enters only as data (a per-(bh, step) 0/1 table plus one shared
in-block causal-triangle tile), so SPMD holds even though each core sees
different masks. Fully-masked below-diagonal chunks are skipped via
suffix-trimmed matmul/exp/mask widths.
"""

import numpy as np

import concourse.bass as bass
import concourse.mybir as mybir
import concourse.tile as tile
from concourse import bacc

B, T, H, D = 2, 2048, 16, 128
BM = 64           # mask block size
NT = T // 128     # 16 k-tiles / q-tiles of 128
NG = 4            # q-groups of 512 queries
BH_PER_CORE = 4
N_CORES = 8
SCALE = float(1.0 / np.sqrt(D))

F16 = mybir.dt.float16
F32 = mybir.dt.float32

# step s enumerates (g, kt): for g in 0..3: for kt in 0..4g+3
STEP_OFF = [0, 4, 12, 24]
N_STEPS = 40

# combined input layout (columns of the per-bh [128, XCOLS] tile)
XQ = 0                      # Q^T  [128, 2048]
XK = XQ + T                 # K^T  [128, 2048]
XV = XK + T                 # V    [128, NT, 128]
XM = XV + NT * 128          # m8   [128, N_STEPS * 16]
XCOLS = XM + N_STEPS * 16   # 6784

LOOKAHEAD = 3


def build_program(loop_n=None):
    nc = bacc.Bacc("TRN2", target_bir_lowering=False, debug=False)

    x_d = nc.dram_tensor(
        "x", (BH_PER_CORE, 128, XCOLS), F16, kind="ExternalInput"
    )
    c01_d = nc.dram_tensor("c01", (128, 128), F16, kind="ExternalInput")
    # o is stored transposed ([d, t] per pair, fp32 unnormalized); host does
    # the final [d, t] -> [t, d] transpose fused with the 1/l normalization
    o_d = nc.dram_tensor("o", (BH_PER_CORE, D, T), F32, kind="ExternalOutput")
    l_d = nc.dram_tensor("l", (BH_PER_CORE, T), F32, kind="ExternalOutput")

    with tile.TileContext(nc) as tc:
        with (
            tc.tile_pool(name="inp", bufs=4) as inp,
            tc.tile_pool(name="const", bufs=1) as cpool,
            tc.tile_pool(name="pt", bufs=6) as ppool,
            tc.tile_pool(name="outp", bufs=3) as opool,
            tc.tile_pool(name="sc", bufs=3, space="PSUM") as scpool,
            tc.tile_pool(name="ol", bufs=1, space="PSUM") as olpool,
        ):
            ones = cpool.tile([128, 1], F16)
            nc.vector.memset(ones[:, :], 1.0)
            c01 = cpool.tile([128, 128], F16)
            nc.sync.dma_start(c01[:, :], c01_d.ap()[:, :])

            if loop_n is not None:
                loop_cm = tc.For_i(
                    0,
                    loop_n,
                    1,
                    hint_engines=(
                        mybir.EngineType.PE,
                        mybir.EngineType.Activation,
                        mybir.EngineType.DVE,
                        mybir.EngineType.SP,
                        mybir.EngineType.Pool,
                    ),
                )
                loop_cm.__enter__()

            tiles = []
            for bh in range(BH_PER_CORE):
                xt = inp.tile([128, XCOLS], F16, tag="xt")
                # split the load so the Q^T/K^T half lands (and compute
                # starts) before the V/mask half streams in
                nc.sync.dma_start(xt[:, :XV], x_d.ap()[bh][:, :XV])
                nc.sync.dma_start(xt[:, XV:], x_d.ap()[bh][:, XV:])
                tiles.append(xt)

            # software pipeline: pending holds the deferred PE consumer ops
            # (O/l matmuls of a step, group evacuations); they are emitted
            # LOOKAHEAD producer-steps later so the PE/DVE never stall on the
            # exp/mask chain of the step they consume
            pending = []

            def flush(limit=0):
                while len(pending) > limit:
                    pending.pop(0)()

            for bh in range(BH_PER_CORE):
                xt = tiles[bh]
                qT = xt[:, XQ : XQ + T]
                kT = xt[:, XK : XK + T]
                v = xt[:, XV : XV + NT * 128].rearrange("p (nt d) -> p nt d", d=128)
                m8 = xt[:, XM : XM + N_STEPS * 16]
                for g in range(NG):
                    nkt = 4 * g + 4
                    # combined accumulator: O^T in cols [0:512] (bank pair
                    # half A), l in partition 0 cols [512:1024] (half B);
                    # single 2-bank tile evacuated with one copy
                    ol = olpool.tile([128, 1024], F32)

                    for kt0 in range(0, nkt, 2):
                        # causal suffix-trim: columns below the diagonal chunk
                        # are fully masked; skip them
                        offs = [
                            max(0, kt0 + h - 4 * g) * 128 if kt0 + h > 4 * g else 0
                            for h in range(2)
                        ]
                        sc = scpool.tile([128, 1024], F32)  # 2 psum banks
                        for h in range(2):
                            kt = kt0 + h
                            o0 = offs[h]
                            nc.tensor.matmul(
                                sc[:, h * 512 + o0 : (h + 1) * 512],
                                lhsT=kT[:, kt * 128 : (kt + 1) * 128],
                                rhs=qT[:, g * 512 + o0 : (g + 1) * 512],
                                start=True,
                                stop=True,
                            )
                        pt = ppool.tile([128, 1024], F16)
                        s0 = STEP_OFF[g] + kt0
                        if offs[0] == 0 and offs[1] == 0:
                            nc.scalar.activation(
                                pt[:, :],
                                sc[:, :],
                                mybir.ActivationFunctionType.Exp,
                                scale=SCALE,
                            )
                            nc.vector.tensor_mul(
                                pt[:, :],
                                pt[:, :],
                                m8[:, s0 * 16 : (s0 + 2) * 16]
                                .rearrange("p (j t) -> p j t", t=2)
                                .broadcast_to([128, 16, 2, 32])
                                .rearrange("p j t r -> p j r t"),
                            )
                        else:
                            for h in range(2):
                                o0 = h * 512 + offs[h]
                                w = 512 - offs[h]
                                nb = w // 64
                                nc.scalar.activation(
                                    pt[:, o0 : o0 + w],
                                    sc[:, o0 : o0 + w],
                                    mybir.ActivationFunctionType.Exp,
                                    scale=SCALE,
                                )
                                nc.vector.tensor_mul(
                                    pt[:, o0 : o0 + w],
                                    pt[:, o0 : o0 + w],
                                    m8[
                                        :,
                                        (s0 + h) * 16 + 2 * (offs[h] // 64)
                                        : (s0 + h + 1) * 16,
                                    ]
                                    .rearrange("p (j t) -> p j t", t=2)
                                    .broadcast_to([128, nb, 2, 32])
                                    .rearrange("p j t r -> p j r t"),
                                )
                        # in-chunk causal triangle on diagonal chunks (on the
                        # otherwise-idle gpsimd engine)
                        for h in range(2):
                            kt = kt0 + h
                            if 4 * g <= kt <= 4 * g + 3:
                                c0 = h * 512 + (kt - 4 * g) * 128
                                nc.gpsimd.tensor_mul(
                                    pt[:, c0 : c0 + 128],
                                    pt[:, c0 : c0 + 128],
                                    c01[:, :],
                                )

                        # deferred consumers of this step's pt: emitted
                        # LOOKAHEAD steps later so the PE stream runs ahead
                        def make_consumer(
                            pt=pt, ol=ol, offs=offs, kt0=kt0, nkt=nkt, v=v
                        ):
                            def consume():
                                for h in range(2):
                                    kt = kt0 + h
                                    o0 = offs[h]
                                    nc.tensor.matmul(
                                        ol[:, o0:512],
                                        lhsT=v[:, kt, :],
                                        rhs=pt[:, h * 512 + o0 : (h + 1) * 512],
                                        start=(kt == 0),
                                        stop=(kt == nkt - 1),
                                    )
                                    nc.tensor.matmul(
                                        ol[0:1, 512 + o0 :],
                                        lhsT=ones[:, :],
                                        rhs=pt[:, h * 512 + o0 : (h + 1) * 512],
                                        start=(kt == 0),
                                        stop=(kt == nkt - 1),
                                    )

                            return consume

                        flush(LOOKAHEAD - 1)
                        pending.append(make_consumer())

                    # single evacuation copy for [O | l]; deferred like a step
                    def make_evac(bh=bh, g=g, ol=ol):
                        def evac():
                            oln = opool.tile([128, 1024], F32, tag="oln")
                            nc.vector.tensor_copy(oln[:, :], ol[:, :])
                            nc.sync.dma_start(
                                o_d.ap()[bh][:, g * 512 : (g + 1) * 512],
                                oln[:, 0:512],
                            )
                            nc.sync.dma_start(
                                l_d.ap()[bh : bh + 1, g * 512 : (g + 1) * 512],
                                oln[0:1, 512:1024],
                            )

                        return evac

                    pending.append(make_evac())

            flush()

            if loop_n is not None:
                loop_cm.__exit__(None, None, None)

    nc.compile()
    return nc


def make_host_inputs(q, k, v, block_mask):
    """Split full inputs into 8 per-core input maps (4 (b,h) pairs each).

    Each pair's inputs are packed into one contiguous [128, XCOLS] fp16
    plane: [ Q^T | K^T | V(t%128 -> partition, 16, d) | m8 mask table ].
    """
    q, k, v = np.asarray(q), np.asarray(k), np.asarray(v)
    block_mask = np.asarray(block_mask)
    pairs = [(b, h) for b in range(B) for h in range(H)]
    kb_idx = np.arange(32)
    vis_causal = kb_idx[:, None] <= kb_idx[None, :]  # [kb, qb]
    c01 = (np.arange(128)[None, :] >= np.arange(128)[:, None]).astype(np.float16)

    in_maps = []
    for c in range(N_CORES):
        sel = pairs[c * BH_PER_CORE : (c + 1) * BH_PER_CORE]
        x = np.zeros((BH_PER_CORE, 128, XCOLS), np.float16)
        for i, (b, h) in enumerate(sel):
            x[i, :, XQ : XQ + T] = q[b, :, h, :].T
            x[i, :, XK : XK + T] = k[b, :, h, :].T
            # V: [t, d] -> [t % 128, t // 128, d]
            x[i, :, XV : XV + NT * 128] = (
                v[b, :, h, :].reshape(NT, 128, D).transpose(1, 0, 2).reshape(128, -1)
            )
            # m8 mask table
            vis = (block_mask[b, h].T & vis_causal).astype(np.float16)
            for g in range(NG):
                for kt in range(4 * g + 4):
                    s = STEP_OFF[g] + kt
                    for half in range(2):
                        kb = 2 * kt + half
                        x[
                            i,
                            half * 64 : (half + 1) * 64,
                            XM + s * 16 : XM + (s + 1) * 16,
                        ] = np.repeat(vis[kb, 8 * g : 8 * g + 8], 2)[None, :]
        in_maps.append({"x": x, "c01": c01})
    return in_maps


_NC_CACHE = {}


def get_program():
    if "nc" not in _NC_CACHE:
        _NC_CACHE["nc"] = build_program()
    return _NC_CACHE["nc"]


def assemble_output(res, inputs=None):
    pairs = [(b, h) for b in range(B) for h in range(H)]
    out = np.zeros((B, T, H, D), np.float16)
    for c in range(N_CORES):
        sel = pairs[c * BH_PER_CORE : (c + 1) * BH_PER_CORE]
        oc = res.results[c]["o"]  # [bh, d, t] transposed-unnormalized fp32
        lc = res.results[c]["l"]
        for i, (b, h) in enumerate(sel):
            out[b, :, h, :] = (oc[i].T / lc[i][:, None]).astype(np.float16)
    return out


def kernel(q, k, v, block_mask, _trace=False):
    from concourse.bass_utils import run_bass_kernel_spmd

    nc = get_program()
    in_maps = make_host_inputs(q, k, v, block_mask)
    res = run_bass_kernel_spmd(
        nc, in_maps, core_ids=list(range(N_CORES)), trace=_trace
    )
    out = assemble_output(res)
    if _trace:
        return out, res
    return out
